# revision 27
# baseline (speedup 1.0000x reference)
"""Trainium2 Bass kernel for windowed 3D attention (nn_Attention_12927851561046).

512 windows of 343-token, 4-head, 32-dim-per-head attention over d=128.
Pure data parallel: 64 windows per core across 8 NeuronCores.

v2 design (vs the eb-multiply baseline):
  - f16 data path everywhere bf16 was used (same matmul/DVE cost, more
    mantissa), f32 psum throughout.
  - rel-pos bias is folded into the sim PSUM *before* the QK matmul via a
    cheap fp8-e4m3 DoubleRow matmul (identity lhsT x bias-table rhs at 0.5
    cycles/row, broadcast rhs so the halved table is added twice). The
    softmax then needs NO per-element bias multiply at all.
  - heads are processed in PSUM pairs (one 2-bank tile per pair):
      pair A (h0,h1): bias-preload + QK accumulate, then ONE batched ACT
        Exp over both banks -> attn f16.
      pair B (h2,h3): QK only, then ONE batched DVE affine_then_add
        (Schraudolph: int16 bits of exp(sim+bias), bias table in f16 rides
        the in1 operand) -> attn f16.
  - PV, rowsum-rides-PV, rep/recip/anrm and the final projection keep the
    baseline structure, all in f16 (rc stays f32 for reciprocal_approx_fast).
    The rep matmuls reuse the (free-at-that-point) ao PSUM banks.
  - Pool only does the anrm multiplies + ones memsets (GPSIMD cannot touch
    PSUM, so it cannot help with the psum->sbuf conversions) and triggers
    the xt input-prefetch DMA on its own queue so the prefetch is never
    head-of-line blocked behind the SP out-DMAs.
  - Two-deep software pipeline: iteration w emits window w's sims/PV,
    window w-1's tail, and window w+1's q/k/v projections, with the next
    window's ACT casts emitted inside the chunk-2 phase.

  TimelineSim: 377653 ns (baseline 435233), rel err 8.75e-3.
"""

import sys
from contextlib import ExitStack

import numpy as np

sys.path.insert(0, "/opt/trn_rl_repo")

import ml_dtypes  # noqa: E402

import concourse.bass as bass  # noqa: E402
import concourse.tile as tile  # noqa: E402
from concourse.tile import add_dep_helper  # noqa: E402
from concourse import bacc, mybir  # noqa: E402
from concourse import bass_utils  # noqa: E402

F16 = mybir.dt.float16
F32 = mybir.dt.float32
I16 = mybir.dt.int16
E4 = mybir.dt.float8e4

NW = 64          # windows per core
N = 343          # tokens per window
D = 128
H = 4
DH = 32
NP = 384         # padded tokens (zeros beyond 343)
JOFF = [0, 128, 256]

# f16-domain Schraudolph exp: bits16 = sim*SCHR_A + SCHR_B (+ f16 leb table)
SCHR_A = 1024.0 / float(np.log(2.0))   # 1477.32
SCHR_B = 15360.0 - 59.0                # rms-calibrated
LEB_MASK = -60000.0                    # masked rows -> bits < 0 -> sat -> -0

TRACE = False
TRACE_KWARGS = {}

_cache = {}


def _build_kernel():
    nc = bacc.Bacc(
        "TRN2",
        target_bir_lowering=False,
        debug=False,
        enable_asserts=False,
        num_devices=8,
    )
    xt_d = nc.dram_tensor("xt", (NW, D, NP), F16, kind="ExternalInput").ap()
    wqkv_d = nc.dram_tensor("wqkv", (D, 3 * D), F16, kind="ExternalInput").ap()
    wout_d = nc.dram_tensor("wout", (2 * 97, D), F16, kind="ExternalInput").ap()
    idid_d = nc.dram_tensor("idid", (D, 2 * D), E4, kind="ExternalInput").ap()
    bias8_d = nc.dram_tensor("bias8", (D, 6 * N), E4, kind="ExternalInput").ap()
    leb_d = nc.dram_tensor("leb", (D, 6 * N), F16, kind="ExternalInput").ap()
    out_d = nc.dram_tensor("out", (NW, N, D), F32, kind="ExternalOutput").ap()

    with tile.TileContext(nc) as tc:
        with ExitStack() as ctx:
            _body(ctx, tc, out_d, xt_d, wqkv_d, wout_d, idid_d, bias8_d, leb_d)

    nc.compile()
    return nc


def _chain(insts):
    for a, b in zip(insts[1:], insts[:-1]):
        add_dep_helper(a.ins, b.ins, sync=False, reason="psum accumulation order")


def _ao_tile(ps, tag):
    return ps.tile([97, N], F32, tag=tag, bufs=1, padded_shape=[97, 512],
                   name=tag)


def _body(ctx, tc, out_d, xt_d, wqkv_d, wout_d, idid_d, bias8_d, leb_d):
    nc = tc.nc

    const = ctx.enter_context(tc.tile_pool(name="const", bufs=1))
    sb = ctx.enter_context(tc.tile_pool(name="sb", bufs=4))
    ps = ctx.enter_context(tc.tile_pool(name="ps", bufs=1, space="PSUM"))

    # constants
    wqkv = const.tile([D, 3 * D], F16)
    nc.sync.dma_start(wqkv[:], wqkv_d[:])
    woutA = const.tile([97, D], F16)
    nc.sync.dma_start(woutA[:], wout_d[0:97, :])
    woutB = const.tile([97, D], F16)
    nc.sync.dma_start(woutB[:], wout_d[97:194, :])
    idid = const.tile([D, 2 * D], E4)
    nc.sync.dma_start(idid[:], idid_d[:])
    bias8 = const.tile([D, 6 * N], E4)
    nc.sync.dma_start(bias8[:], bias8_d[:])
    leb = const.tile([D, 6 * N], F16)
    nc.sync.dma_start(leb[:], leb_d[:])
    # rowsum-replication indicator: row 32 (rs of even head) -> out rows
    # 0..63, row 64 (rs of odd head) -> out rows 64..96; all other rows zero.
    ind97 = const.tile([97, 97], F16)
    nc.vector.memset(ind97[:], 0.0)
    nc.vector.memset(ind97[32:33, 0:64], 1.0)
    nc.vector.memset(ind97[64:65, 64:97], 1.0)

    zrow = const.tile([1, 97], F16)
    nc.vector.memset(zrow[:], 0.0)

    # one-time zero of the ao banks (rows 33..63 are never written by the
    # per-window matmuls but are read by the rsbf cast)
    zA = _ao_tile(ps, "aoA")
    zB = _ao_tile(ps, "aoB")
    nc.tensor.matmul(zA[:, 0:N], lhsT=zrow[:], rhs=leb[0:1, 0:N],
                     start=True, stop=True)
    nc.tensor.matmul(zB[:, 0:N], lhsT=zrow[:], rhs=leb[0:1, 0:N],
                     start=True, stop=True)

    def emit_xt(w):
        # Pool-triggered DMA: separate queue from the SP out-DMAs, so the
        # input prefetch is never head-of-line blocked behind them.
        xt = sb.tile([D, NP], F16, tag="xt", name="xt")
        nc.gpsimd.dma_start(xt[:], xt_d[w])
        return xt

    def emit_qk(xt):
        qp = ps.tile([D, N], F32, tag="qk", bufs=1, padded_shape=[D, 512], name="qp")
        nc.tensor.matmul(qp[:], lhsT=wqkv[:, 0:D], rhs=xt[:, 0:N], start=True, stop=True)
        qsb = sb.tile([D, N], F16, tag="qsb", name="qsb")
        nc.scalar.copy(qsb[:], qp[:])                 # ACT cast
        kp = ps.tile([D, NP], F32, tag="qk", bufs=1, padded_shape=[D, 512], name="kp")
        nc.tensor.matmul(kp[:], lhsT=wqkv[:, D:2 * D], rhs=xt[:], start=True, stop=True)
        ksb = sb.tile([D, NP], F16, tag="ksb", name="ksb")
        nc.scalar.copy(ksb[:], kp[:])                 # ACT cast
        return qsb, ksb

    def emit_vp(xt):
        vp = ps.tile([D, 3 * D], F32, tag="qk", bufs=1, padded_shape=[D, 512], name="vp")
        v_mms = []
        for c in range(3):
            v_mms.append(nc.tensor.matmul(
                vp[:, c * D:(c + 1) * D],
                lhsT=xt[:, JOFF[c]:JOFF[c] + D],
                rhs=wqkv[:, 2 * D:3 * D],
                start=(c == 0), stop=(c == 2),
            ))
        _chain(v_mms)

        vsb1 = sb.tile([D, 3 * 132], F16, tag="vsb1", name="vsb1")
        vdst = vsb1[:].rearrange("p (c g s) -> p c g s", c=3, g=2)   # s=66
        vsrc = vp[:].rearrange("p (c g s) -> p c g s", c=3, g=2)     # s=64
        nc.vector.tensor_copy(vdst[:, :, :, 0:32], vsrc[:, :, :, 0:32])
        nc.vector.tensor_copy(vdst[:, :, :, 34:66], vsrc[:, :, :, 32:64])
        nc.gpsimd.memset(vdst[:, :, :, 32:34], 1.0)  # ones cols on Pool
        return vsb1

    def act_heads(c):
        return (0, 1)

    def emit_sims_mms(qsb, ksb, c):
        """ACT pair: DR bias preload + QK matmuls -> batched Exp.
        DVE pair: QK matmuls only -> batched Schraudolph."""
        attn = sb.tile([D, H * N], F16, tag="attn", bufs=8, name="attn")
        hA = act_heads(c)

        pA = ps.tile([D, 1024], F32, tag="simA", bufs=1,
                     padded_shape=[D, 1024], name="pA")
        mms = []
        for i, h in enumerate(hA):
            sl = pA[:, 512 * i:512 * i + N]
            k = 2 * c + i
            mms.append(nc.tensor.matmul(
                sl,
                lhsT=idid[:].rearrange("p (t m) -> p t m", t=2),
                rhs=bias8[:, k * N:(k + 1) * N]
                    .rearrange("p (t n) -> p t n", t=1)
                    .broadcast_to([D, 2, N]),
                start=True, stop=False,
                perf_mode=mybir.MatmulPerfMode.DoubleRow,
                skip_group_check=True,
            ))
            mms.append(nc.tensor.matmul(
                sl,
                lhsT=ksb[DH * h:DH * (h + 1), JOFF[c]:JOFF[c] + D],
                rhs=qsb[DH * h:DH * (h + 1), 0:N],
                tile_position=(DH * h, 0),
                start=False, stop=True,
                skip_group_check=True,
            ))
        _chain(mms)

        pB = ps.tile([D, 1024], F32, tag="simB", bufs=1,
                     padded_shape=[D, 1024], name="pB")
        hB = (0, 1) if hA == (2, 3) else (2, 3)
        for i, h in enumerate(hB):
            sl = pB[:, 512 * i:512 * i + N]
            nc.tensor.matmul(
                sl,
                lhsT=ksb[DH * h:DH * (h + 1), JOFF[c]:JOFF[c] + D],
                rhs=qsb[DH * h:DH * (h + 1), 0:N],
                tile_position=(DH * h, 0),
                start=True, stop=True,
            )
        return pA, pB, attn

    def emit_expA(pA, attn, c):
        hA = act_heads(c)
        nc.scalar.activation(
            attn[:, hA[0] * N:(hA[0] + 2) * N].rearrange("p (t n) -> p t n", t=2),
            pA[:].rearrange("p (t n) -> p t n", t=2)[:, :, 0:N],
            mybir.ActivationFunctionType.Exp,
        )

    def emit_affB(pB, attn, c):
        hB = (0, 1) if act_heads(c) == (2, 3) else (2, 3)
        nc.vector.affine_then_add(
            attn[:, hB[0] * N:(hB[0] + 2) * N].bitcast(I16)
                .rearrange("p (t n) -> p t n", t=2),
            pB[:].rearrange("p (t n) -> p t n", t=2)[:, :, 0:N],
            leb[:, 2 * c * N:(2 * c + 2) * N].rearrange("p (t n) -> p t n", t=2),
            SCHR_A, SCHR_B,
        )

    def emit_chunk_aos(aoA, aoB, vsb1, attn, c, ao_mms):
        for h in (0, 1, 2, 3):
            bank = aoA if h < 2 else aoB
            off = 64 * (h % 2)
            ao_mms.append(nc.tensor.matmul(
                bank[off:off + 33, :],
                lhsT=vsb1[:, 132 * c + 33 * h:132 * c + 33 * h + 33],
                rhs=attn[:, N * h:N * (h + 1)],
                tile_position=(0, off),
                start=(c == 0), stop=(c == 2),
                skip_group_check=True,
            ))

    def emit_rsbf(aoA, aoB):
        rsbf = sb.tile([97, 2 * N], F16, tag="rsbf", name="rsbf")
        nc.scalar.copy(rsbf[:, 0:N], aoA[:, 0:N])
        nc.scalar.copy(rsbf[:, N:2 * N], aoB[:, 0:N])
        return rsbf

    def emit_rep(rsbf, half, nm):
        rep = ps.tile([97, N], F32, tag="aoA" if half == 0 else "aoB", bufs=1,
                      padded_shape=[97, 512], name=nm)
        nc.tensor.matmul(rep[:], lhsT=ind97[:], rhs=rsbf[:, half * N:(half + 1) * N],
                         start=True, stop=True)
        rc = sb.tile([97, N], F32, tag=nm + "rc", name=nm + "rc")
        nc.vector.reciprocal_approx_fast(rc[:], rep[:])
        return rc

    def emit_anrm(rsbf, rc, half, nm):
        anrm = sb.tile([97, N], F16, tag=nm, name=nm)
        nc.gpsimd.tensor_mul(anrm[:], rsbf[:, half * N:(half + 1) * N], rc[:])
        return anrm

    def emit_fin(w, anrmA, anrmB):
        fp = ps.tile([D, 3 * D], F32, tag="fin", bufs=1, padded_shape=[D, 512],
                     name="fp")
        f_mms = []
        for c in range(3):
            jc = min(D, N - JOFF[c])
            f_mms.append(nc.tensor.matmul(
                fp[0:jc, c * D:(c + 1) * D],
                lhsT=anrmA[:, JOFF[c]:JOFF[c] + jc],
                rhs=woutA[:],
                start=True, stop=False,
                skip_group_check=True,
            ))
            f_mms.append(nc.tensor.matmul(
                fp[0:jc, c * D:(c + 1) * D],
                lhsT=anrmB[:, JOFF[c]:JOFF[c] + jc],
                rhs=woutB[:],
                start=False, stop=True,
                skip_group_check=True,
            ))
        _chain(f_mms)

        fsb = sb.tile([D, 3 * D], F32, tag="fsb", name="fsb")
        cp1 = nc.scalar.copy(fsb[:, 0:2 * D], fp[:, 0:2 * D])
        add_dep_helper(cp1.ins, f_mms[-1].ins, sync=True,
                       reason="read after accumulation group closes")
        nc.vector.tensor_copy(fsb[0:87, 2 * D:3 * D], fp[0:87, 2 * D:3 * D])

        dst01 = out_d[w, 0:256, :].rearrange("(c p) d -> p c d", p=D)
        src01 = fsb[:, 0:256].rearrange("p (c d) -> p c d", c=2)
        nc.sync.dma_start(dst01, src01)
        nc.sync.dma_start(out_d[w, 256:343, :], fsb[0:87, 2 * D:3 * D])

    # two-deep software pipeline: iteration w emits window w's sims/PV,
    # window w-1's normalize/projection tail, and window w+1's q/k/v head.
    # The head casts are interleaved into the chunk phases so the ACT/DVE
    # queues have them ready before the next iteration's sim matmuls.
    prev = None   # (w, rsbf) of the previous window
    xt_cur = emit_xt(0)
    qsb, ksb = emit_qk(xt_cur)
    vsb1 = emit_vp(xt_cur)
    xt_next = emit_xt(1)
    for w in range(NW):
        if prev is not None:
            rcA = emit_rep(prev[1], 0, "r1")
        aoA = _ao_tile(ps, "aoA")
        aoB = _ao_tile(ps, "aoB")
        ao_mms = []
        pA0, pB0, attn0 = emit_sims_mms(qsb, ksb, 0)
        emit_expA(pA0, attn0, 0)
        emit_affB(pB0, attn0, 0)
        if prev is not None:
            rcB = emit_rep(prev[1], 1, "r2")
            anrmA = emit_anrm(prev[1], rcA, 0, "anrmA")
        pA1, pB1, attn1 = emit_sims_mms(qsb, ksb, 1)
        emit_expA(pA1, attn1, 1)
        emit_affB(pB1, attn1, 1)
        emit_chunk_aos(aoA, aoB, vsb1, attn0, 0, ao_mms)
        if prev is not None:
            anrmB = emit_anrm(prev[1], rcB, 1, "anrmB")
        last = w + 1 >= NW
        pA2, pB2, attn2 = emit_sims_mms(qsb, ksb, 2)
        if not last:
            qsb_n, ksb_n = emit_qk(xt_next)   # ACT casts land before exp2
        emit_expA(pA2, attn2, 2)
        emit_affB(pB2, attn2, 2)
        emit_chunk_aos(aoA, aoB, vsb1, attn1, 1, ao_mms)
        if prev is not None:
            emit_fin(prev[0], anrmA, anrmB)
        emit_chunk_aos(aoA, aoB, vsb1, attn2, 2, ao_mms)
        _chain(ao_mms)
        if not last:
            vsb1_n = emit_vp(xt_next)
        rsbf = emit_rsbf(aoA, aoB)
        prev = (w, rsbf)
        if not last:
            qsb, ksb, vsb1 = qsb_n, ksb_n, vsb1_n
            xt_next = emit_xt(w + 2) if w + 2 < NW else None

    # drain the last window's tail
    rcA = emit_rep(prev[1], 0, "r1")
    rcB = emit_rep(prev[1], 1, "r2")
    anrmA = emit_anrm(prev[1], rcA, 0, "anrmA")
    anrmB = emit_anrm(prev[1], rcB, 1, "anrmB")
    emit_fin(prev[0], anrmA, anrmB)


def _prep_inputs(x, w_qkv, w_out, bias_table, rel_idx):
    x = np.asarray(x, dtype=np.float32)
    w_qkv = np.asarray(w_qkv, dtype=np.float32)
    w_out = np.asarray(w_out, dtype=np.float32)
    bias_table = np.asarray(bias_table, dtype=np.float32)
    rel_idx = np.asarray(rel_idx)

    scale = DH ** -0.5
    wq = w_qkv[:, 0:D] * scale
    wqkv_s = np.concatenate([wq, w_qkv[:, D:3 * D]], axis=1)
    wqkv_f16 = wqkv_s.astype(np.float16)

    # wout97: per bank [h_even rows | zero gap | rs row zero | h_odd rows]
    wout97 = np.zeros((194, D), dtype=np.float32)
    wout97[0:32] = w_out[0:32]          # h0 (anrm rows 0..31)
    wout97[65:97] = w_out[32:64]        # h1 (anrm rows 65..96)
    wout97[97:129] = w_out[64:96]       # h2
    wout97[162:194] = w_out[96:128]     # h3
    wout97_f16 = wout97.astype(np.float16)

    xr = x.reshape(8 * 64, N, D)
    xtf = np.zeros((8 * 64, D, NP), dtype=np.float32)
    xtf[:, :, 0:N] = xr.transpose(0, 2, 1)
    xt = xtf.astype(np.float16).reshape(8, NW, D, NP)

    # idid: two identity matrices side by side (DoubleRow k-tiles)
    idid = np.zeros((D, 2 * D), dtype=np.float32)
    idid[:, 0:D] = np.eye(D)
    idid[:, D:2 * D] = np.eye(D)
    idid_e4 = idid.astype(ml_dtypes.float8_e4m3)

    bias = bias_table[rel_idx]                     # (i, j, h)
    biasT = bias.transpose(1, 2, 0)                # (j, h, i)

    # bias8: halved bias tables for pair-A heads (h0,h1), slot k = 2c+h.
    # The DR preload adds the table twice (broadcast k-tiles), restoring the
    # full bias. Rows beyond j=343 (chunk 2) get -150 -> psum -300 -> exp 0.
    b8 = np.full((D, 6 * N), -150.0, dtype=np.float32)
    for c in range(3):
        jn = min(D, N - JOFF[c])
        hA = (0, 1)                            # ACT-pair heads per chunk
        for i, h in enumerate(hA):
            k = 2 * c + i
            b8[0:jn, k * N:(k + 1) * N] = biasT[JOFF[c]:JOFF[c] + jn, h, :] / 2.0
    bias8_e4 = b8.astype(ml_dtypes.float8_e4m3)

    # leb: f16 Schraudolph bias-fold tables for pair-B heads (h2,h3),
    # slot k = 2c+(h-2); masked (padded-j) rows get LEB_MASK.
    lb = np.full((D, 6 * N), LEB_MASK, dtype=np.float32)
    for c in range(3):
        jn = min(D, N - JOFF[c])
        hB = (2, 3)                            # DVE-pair heads per chunk
        for i, h in enumerate(hB):
            k = 2 * c + i
            lb[0:jn, k * N:(k + 1) * N] = SCHR_A * biasT[JOFF[c]:JOFF[c] + jn, h, :]
    leb_f16 = lb.astype(np.float16)

    in_maps = []
    for core in range(8):
        in_maps.append({
            "xt": np.ascontiguousarray(xt[core]),
            "wqkv": wqkv_f16,
            "wout": wout97_f16,
            "idid": idid_e4,
            "bias8": bias8_e4,
            "leb": leb_f16,
        })
    return in_maps


def kernel(x, w_qkv, w_out, bias_table, rel_idx):
    if "nc" not in _cache:
        _cache["nc"] = _build_kernel()
    nc = _cache["nc"]
    in_maps = _prep_inputs(x, w_qkv, w_out, bias_table, rel_idx)
    res = bass_utils.run_bass_kernel_spmd(
        nc, in_maps, core_ids=list(range(8)), trace=TRACE, **TRACE_KWARGS
    )
    _cache["last_result"] = res
    outs = [res.results[c]["out"] for c in range(8)]
    full = np.concatenate(outs, axis=0)             # (512, 343, 128)
    return full.reshape(1, 8, 8, 8, 7, 7, 7, D).astype(np.float32)
